# revision 8
# baseline (speedup 1.0000x reference)
"""Trainium2 Bass kernel for nn_BaseTBLoRa (moe_routing).

out[b,s,:] = x[b,s,:] @ W_base.T + b_base + 2.0 * ((x[b,s,:] @ A_w[e_b].T) @ B_w[e_b].T)
with e_b = segment[b].  B=8, S=2048, D=1024, Do=1024, R=16, E=8.

Sharding: data-parallel over batch — core b handles batch b (B == n_cores).

Two key moves vs the original baseline:

1. LoRA fold (host): each core serves exactly one expert, so
       W_eff = W_base + LORA_SCALE * (B_w[e] @ A_w[e])     (rank-16 update)
   making the device kernel a pure dense GEMM out = x @ W_eff.T + b_base.
   256 N=512 bf16 PE streams/core remain (the FLOP lower bound on the
   128x128 PE; fp8 was measured at rel_err 3.1e-2 > 2e-2 tolerance).

2. Two-ring DMA plan: HWDGE DMAs are FIFO per issuing engine and the
   sequencer blocks on data-ready sem waits BEFORE enqueuing, so a store
   that waits on compute blocks every later load on the same ring.
   Loads therefore free-run on qSPDynamicHW (nc.sync: 4x 1MB x chunks,
   issued up front) while weights + stores go on qActDynamicHW
   (nc.scalar: W 2MB, bias, then one 1MB store per chunk).  Coarse
   transfers (>=1MB) run at ~78% of the 358 GB/s peak vs ~30% for the
   per-dt 128KB slices the old kernel used.

Per s-tile (128 tokens): 16 accumulating matmuls (8 dt x 2 N-halves) into a
[128, DO] f32 PSUM pair, then one DVE tensor_add (psum + bias -> bf16 SBUF)
fusing the bias add, the f32->bf16 down-convert, and the PSUM drain.
"""

import ml_dtypes
import numpy as np

import concourse.tile as tile
from concourse import bacc, mybir
from concourse.bass_utils import run_bass_kernel_spmd

LORA_SCALE = 32.0 / 16.0

B, S, D, DO, R = 8, 2048, 1024, 1024, 16
NDT = D // 128   # 8 contraction tiles
NSC = 4          # s macro-chunks
SC = S // NSC    # 512 s per macro-chunk
SUB = SC // 128  # 4 s-tiles per macro-chunk
N_CORES = 8

F32 = mybir.dt.float32
BF16 = mybir.dt.bfloat16

last_in_maps = None
last_results = None


def _build(loop_n=0, probe=None):
    """loop_n > 0 wraps the body in a dynamic For_i (used only for dilation
    timing); the graded path uses loop_n=0 (straight-line program).

    probe='nodma' strips the big DMAs (PE/DVE critical path only);
    probe='nomm'  strips the matmuls/DVE (DMA bandwidth only)."""
    import contextlib

    nc = bacc.Bacc("TRN2", target_bir_lowering=False, debug=False)

    x_d = nc.dram_tensor("x5", [NSC, 128, NDT, SC], BF16, kind="ExternalInput")
    w_d = nc.dram_tensor("wt", [128, NDT, DO], BF16, kind="ExternalInput")
    bias_d = nc.dram_tensor("bias", [128, DO], F32, kind="ExternalInput")
    out_d = nc.dram_tensor("out", [NSC, 128, SUB, DO], BF16, kind="ExternalOutput")

    with tile.TileContext(nc) as tc:
        with (
            tc.tile_pool(name="wpool", bufs=2) as wpool,
            tc.tile_pool(name="cpool", bufs=2) as cpool,
            tc.tile_pool(name="xpool", bufs=5) as xpool,
            tc.tile_pool(name="opool", bufs=2) as opool,
            tc.tile_pool(name="psy", bufs=6, space="PSUM") as psy,
        ):
            loop_cm = tc.For_i(0, loop_n, 1) if loop_n else contextlib.nullcontext()
            nodma = probe in ("nodma", "pemax", "pedve")

            def body():
                # All loads ride the load ring (qSP, nc.sync) in the order the
                # PE consumes them: W.h0, x0, W.h1, bias, x1..x3.  Stores get
                # qAct (nc.scalar) to themselves: a store waits on compute at
                # the ACT sequencer, and with nothing queued behind it on that
                # ring it can never block a load (HWDGE rings are FIFO per
                # issuing engine, and the sequencer blocks BEFORE enqueuing).
                bias_t = cpool.tile([128, DO], F32)
                w_t = wpool.tile([128, NDT, DO], BF16)
                x_ts = []
                if nodma:
                    nc.vector.memset(w_t[:], 0)
                    nc.vector.memset(bias_t[:], 0)
                    if probe == "nodma":
                        for sc in range(NSC):
                            x_t = xpool.tile([128, NDT, SC], BF16)
                            nc.vector.memset(x_t[:], 0)
                            x_ts.append(x_t)
                    else:
                        # pemax/pedve: one shared garbage tile is enough
                        x_t = xpool.tile([128, NDT, SC], BF16)
                        nc.vector.memset(x_t[:], 0)
                        x_ts = [x_t] * NSC
                else:
                    nc.sync.dma_start(w_t[:, 0:NDT // 2, :], w_d[:, 0:NDT // 2, :])
                    for sc in range(NSC):
                        x_t = xpool.tile([128, NDT, SC], BF16)
                        if probe == "noxdma":
                            nc.vector.memset(x_t[:], 0)
                        else:
                            nc.sync.dma_start(x_t[:], x_d[sc])
                        x_ts.append(x_t)
                        if sc == 0:
                            nc.sync.dma_start(
                                w_t[:, NDT // 2:, :], w_d[:, NDT // 2:, :]
                            )
                            nc.sync.dma_start(bias_t[:], bias_d[:])

                for sc in range(NSC):
                    x_t = x_ts[sc]
                    if probe == "nomm":
                        # store straight from x_t to keep loads live
                        nc.scalar.dma_start(out_d[sc], x_t[:])
                        continue

                    o_c = opool.tile([128, SUB, DO], BF16)
                    for sub in range(SUB):
                        # one single-bank PSUM group (8 MMs) per N-half:
                        # finer drain granularity keeps the PE gapless
                        for h in range(2):
                            ps = psy.tile([128, 512], F32)
                            for dt in range(NDT):
                                xt = x_t[:, dt, sub * 128:(sub + 1) * 128]
                                nc.tensor.matmul(
                                    ps[:], xt, w_t[:, dt, h * 512:(h + 1) * 512],
                                    start=(dt == 0), stop=(dt == NDT - 1),
                                )
                            if probe != "pemax":
                                # fused: bias add + f32->bf16 + PSUM drain
                                nc.vector.tensor_add(
                                    o_c[:, sub, h * 512:(h + 1) * 512],
                                    ps[:], bias_t[:, h * 512:(h + 1) * 512],
                                )

                    if probe in ("nodma", "nostore", "pedve"):
                        # tiny keep-alive store — on the store ring so it can
                        # never block loads
                        nc.scalar.dma_start(out_d[sc][:, 0:1, 0:16], o_c[:, 0:1, 0:16])
                    elif probe == "pemax":
                        nc.scalar.dma_start(out_d[sc][:, 0:1, 0:16], x_t[:, 0:1, 0:16])
                    else:
                        nc.scalar.dma_start(out_d[sc], o_c[:])

            # A For_i body may not touch tiles allocated outside the loop, so
            # in timing mode everything moves inside.
            if not loop_n:
                body()
            else:
                with loop_cm:
                    body()

    nc.compile()
    return nc


def _fold_wt(e, W_base, b_base, A_w, B_w):
    W_eff = W_base + LORA_SCALE * (B_w[e] @ A_w[e])
    # wt[p, dt, o] = W_eff[o, dt*128 + p]
    return np.ascontiguousarray(
        W_eff.T.reshape(NDT, 128, DO).transpose(1, 0, 2)
    ).astype(ml_dtypes.bfloat16)


def _prep_core_inputs(x_b, e, W_base, b_base, A_w, B_w, wt=None):
    xT = x_b.T.reshape(NDT, 128, NSC, SC).transpose(2, 1, 0, 3).astype(
        ml_dtypes.bfloat16
    )
    if wt is None:
        wt = _fold_wt(e, W_base, b_base, A_w, B_w)
    bias = np.ascontiguousarray(
        np.broadcast_to(b_base.astype(np.float32), (128, DO))
    )
    return {"x5": xT, "wt": wt, "bias": bias}


def _unshard(raw):
    # out[sc, p, sub, o] holds token sc*512 + sub*128 + p
    return (
        np.asarray(raw)
        .transpose(0, 2, 1, 3)
        .reshape(S, DO)
        .astype(np.float32)
    )


def kernel(x, segment, W_base, b_base, A_w, B_w, _sim=False):
    global last_in_maps, last_results

    x = np.asarray(x, dtype=np.float32)
    W_base = np.asarray(W_base, dtype=np.float32)
    b_base = np.asarray(b_base, dtype=np.float32)
    A_w = np.asarray(A_w, dtype=np.float32)
    B_w = np.asarray(B_w, dtype=np.float32)
    seg = np.asarray(segment).astype(np.int64)

    wt_cache = {}
    for b in range(B):
        e = int(seg[b])
        if e not in wt_cache:
            wt_cache[e] = _fold_wt(e, W_base, b_base, A_w, B_w)

    in_maps = [
        _prep_core_inputs(
            x[b], int(seg[b]), W_base, b_base, A_w, B_w, wt=wt_cache[int(seg[b])]
        )
        for b in range(B)
    ]
    last_in_maps = in_maps

    nc = _build()

    if _sim:
        from concourse.bass_interp import CoreSim

        outs = []
        for b in range(B):
            sim = CoreSim(nc)
            for name, arr in in_maps[b].items():
                sim.tensor(name)[:] = arr
            sim.simulate()
            outs.append(_unshard(sim.tensor("out")))
        return np.stack(outs)

    res = run_bass_kernel_spmd(nc, in_maps, list(range(N_CORES)))
    last_results = res
    return np.stack([_unshard(res.results[c]["out"]) for c in range(N_CORES)])


# revision 11
# speedup vs baseline: 1.2102x; 1.2102x over previous
"""Trainium2 Bass kernel for nn_BaseTBLoRa (moe_routing).

out[b,s,:] = x[b,s,:] @ W_base.T + b_base + 2.0 * ((x[b,s,:] @ A_w[e_b].T) @ B_w[e_b].T)
with e_b = segment[b].  B=8, S=2048, D=1024, Do=1024, R=16, E=8.

Sharding: data-parallel over batch — core b handles batch b (B == n_cores).

Two key moves vs the original baseline:

1. LoRA fold (host): each core serves exactly one expert, so
       W_eff = W_base + LORA_SCALE * (B_w[e] @ A_w[e])     (rank-16 update)
   making the device kernel a pure dense GEMM out = x @ W_eff.T + b_base.
   256 N=512 bf16 PE streams/core remain (the FLOP lower bound on the
   128x128 PE; fp8 was measured at rel_err 3.1e-2 > 2e-2 tolerance).

2. Two-ring DMA plan: HWDGE DMAs are FIFO per issuing engine and the
   sequencer blocks on data-ready sem waits BEFORE enqueuing, so a store
   that waits on compute blocks every later load on the same ring.
   Loads therefore free-run on qSPDynamicHW (nc.sync: 4x 1MB x chunks,
   issued up front) while weights + stores go on qActDynamicHW
   (nc.scalar: W 2MB, bias, then one 1MB store per chunk).  Coarse
   transfers (>=1MB) run at ~78% of the 358 GB/s peak vs ~30% for the
   per-dt 128KB slices the old kernel used.

Per s-tile (128 tokens): 16 accumulating matmuls (8 dt x 2 N-halves) into a
[128, DO] f32 PSUM pair, then one DVE tensor_add (psum + bias -> bf16 SBUF)
fusing the bias add, the f32->bf16 down-convert, and the PSUM drain.
"""

import ml_dtypes
import numpy as np

import concourse.tile as tile
from concourse import bacc, mybir
from concourse.bass_utils import run_bass_kernel_spmd

LORA_SCALE = 32.0 / 16.0

B, S, D, DO, R = 8, 2048, 1024, 1024, 16
NDT = D // 128   # 8 contraction tiles
NSC = 4          # s macro-chunks
SC = S // NSC    # 512 s per macro-chunk
SUB = SC // 128  # 4 s-tiles per macro-chunk
N_CORES = 8

F32 = mybir.dt.float32
BF16 = mybir.dt.bfloat16

last_in_maps = None
last_results = None


def _build(loop_n=0, probe=None, psy_bufs=4, loads_on="sp"):
    """loop_n > 0 wraps the body in a dynamic For_i (used only for dilation
    timing); the graded path uses loop_n=0 (straight-line program).

    probe='nodma' strips the big DMAs (PE/DVE critical path only);
    probe='nomm'  strips the matmuls/DVE (DMA bandwidth only)."""
    import contextlib

    nc = bacc.Bacc("TRN2", target_bir_lowering=False, debug=False)

    x_d = nc.dram_tensor("x5", [NSC, 128, NDT, SC], BF16, kind="ExternalInput")
    w_d = nc.dram_tensor("wt", [128, NDT, DO], BF16, kind="ExternalInput")
    bias_d = nc.dram_tensor("bias", [128, DO], F32, kind="ExternalInput")
    out_d = nc.dram_tensor("out", [NSC, 128, SUB, DO], BF16, kind="ExternalOutput")

    with tile.TileContext(nc) as tc:
        with (
            tc.tile_pool(name="wpool", bufs=2) as wpool,
            tc.tile_pool(name="cpool", bufs=2) as cpool,
            tc.tile_pool(name="xpool", bufs=5) as xpool,
            tc.tile_pool(name="opool", bufs=2) as opool,
            tc.tile_pool(name="psy", bufs=psy_bufs, space="PSUM") as psy,
        ):
            loop_cm = tc.For_i(0, loop_n, 1) if loop_n else contextlib.nullcontext()
            nodma = probe in ("nodma", "pemax", "pedve")
            ldeng = nc.sync if loads_on == "sp" else nc.scalar

            def body():
                # All loads ride the load ring (qSP, nc.sync) in the order the
                # PE consumes them: W.h0, x0, W.h1, bias, x1..x3.  Stores get
                # qAct (nc.scalar) to themselves: a store waits on compute at
                # the ACT sequencer, and with nothing queued behind it on that
                # ring it can never block a load (HWDGE rings are FIFO per
                # issuing engine, and the sequencer blocks BEFORE enqueuing).
                bias_t = cpool.tile([128, DO], F32)
                w_t = wpool.tile([128, NDT, DO], BF16)
                x_ts = []
                if nodma:
                    nc.vector.memset(w_t[:], 0)
                    nc.vector.memset(bias_t[:], 0)
                    if probe == "nodma":
                        for sc in range(NSC):
                            x_t = xpool.tile([128, NDT, SC], BF16)
                            nc.vector.memset(x_t[:], 0)
                            x_ts.append(x_t)
                    else:
                        # pemax/pedve: one shared garbage tile is enough
                        x_t = xpool.tile([128, NDT, SC], BF16)
                        nc.vector.memset(x_t[:], 0)
                        x_ts = [x_t] * NSC
                else:
                    ldeng.dma_start(w_t[:, 0:NDT // 2, :], w_d[:, 0:NDT // 2, :])
                    for sc in range(NSC):
                        x_t = xpool.tile([128, NDT, SC], BF16)
                        if probe == "noxdma":
                            nc.vector.memset(x_t[:], 0)
                        else:
                            nc.sync.dma_start(x_t[:], x_d[sc])
                        x_ts.append(x_t)
                        if sc == 0:
                            ldeng.dma_start(
                                w_t[:, NDT // 2:, :], w_d[:, NDT // 2:, :]
                            )
                            ldeng.dma_start(bias_t[:], bias_d[:])

                for sc in range(NSC):
                    x_t = x_ts[sc]
                    if probe == "nomm":
                        # store straight from x_t to keep loads live
                        nc.scalar.dma_start(out_d[sc], x_t[:])
                        continue

                    o_c = opool.tile([128, SUB, DO], BF16)
                    for sub in range(SUB):
                        # one single-bank PSUM group (8 MMs) per N-half:
                        # finer drain granularity keeps the PE gapless
                        for h in range(2):
                            ps = psy.tile([128, 512], F32)
                            for dt in range(NDT):
                                xt = x_t[:, dt, sub * 128:(sub + 1) * 128]
                                nc.tensor.matmul(
                                    ps[:], xt, w_t[:, dt, h * 512:(h + 1) * 512],
                                    start=(dt == 0), stop=(dt == NDT - 1),
                                )
                            if probe != "pemax":
                                # fused: bias add + f32->bf16 + PSUM drain
                                nc.vector.tensor_add(
                                    o_c[:, sub, h * 512:(h + 1) * 512],
                                    ps[:], bias_t[:, h * 512:(h + 1) * 512],
                                )

                    if probe in ("nodma", "nostore", "pedve"):
                        # tiny keep-alive store — on the store ring so it can
                        # never block loads
                        nc.scalar.dma_start(out_d[sc][:, 0:1, 0:16], o_c[:, 0:1, 0:16])
                    elif probe == "pemax":
                        nc.scalar.dma_start(out_d[sc][:, 0:1, 0:16], x_t[:, 0:1, 0:16])
                    else:
                        nc.scalar.dma_start(out_d[sc], o_c[:])

            # A For_i body may not touch tiles allocated outside the loop, so
            # in timing mode everything moves inside.
            if not loop_n:
                body()
            else:
                with loop_cm:
                    body()

    nc.compile()
    return nc


def _fold_wt(e, W_base, b_base, A_w, B_w):
    W_eff = W_base + LORA_SCALE * (B_w[e] @ A_w[e])
    # wt[p, dt, o] = W_eff[o, dt*128 + p]
    return np.ascontiguousarray(
        W_eff.T.reshape(NDT, 128, DO).transpose(1, 0, 2)
    ).astype(ml_dtypes.bfloat16)


def _prep_core_inputs(x_b, e, W_base, b_base, A_w, B_w, wt=None):
    xT = x_b.T.reshape(NDT, 128, NSC, SC).transpose(2, 1, 0, 3).astype(
        ml_dtypes.bfloat16
    )
    if wt is None:
        wt = _fold_wt(e, W_base, b_base, A_w, B_w)
    bias = np.ascontiguousarray(
        np.broadcast_to(b_base.astype(np.float32), (128, DO))
    )
    return {"x5": xT, "wt": wt, "bias": bias}


def _unshard(raw):
    # out[sc, p, sub, o] holds token sc*512 + sub*128 + p
    return (
        np.asarray(raw)
        .transpose(0, 2, 1, 3)
        .reshape(S, DO)
        .astype(np.float32)
    )


def kernel(x, segment, W_base, b_base, A_w, B_w, _sim=False):
    global last_in_maps, last_results

    x = np.asarray(x, dtype=np.float32)
    W_base = np.asarray(W_base, dtype=np.float32)
    b_base = np.asarray(b_base, dtype=np.float32)
    A_w = np.asarray(A_w, dtype=np.float32)
    B_w = np.asarray(B_w, dtype=np.float32)
    seg = np.asarray(segment).astype(np.int64)

    wt_cache = {}
    for b in range(B):
        e = int(seg[b])
        if e not in wt_cache:
            wt_cache[e] = _fold_wt(e, W_base, b_base, A_w, B_w)

    in_maps = [
        _prep_core_inputs(
            x[b], int(seg[b]), W_base, b_base, A_w, B_w, wt=wt_cache[int(seg[b])]
        )
        for b in range(B)
    ]
    last_in_maps = in_maps

    nc = _build()

    if _sim:
        from concourse.bass_interp import CoreSim

        outs = []
        for b in range(B):
            sim = CoreSim(nc)
            for name, arr in in_maps[b].items():
                sim.tensor(name)[:] = arr
            sim.simulate()
            outs.append(_unshard(sim.tensor("out")))
        return np.stack(outs)

    res = run_bass_kernel_spmd(nc, in_maps, list(range(N_CORES)))
    last_results = res
    return np.stack([_unshard(res.results[c]["out"]) for c in range(N_CORES)])
